# revision 5
# baseline (speedup 1.0000x reference)
"""Trainium2 Bass kernel for nn_Basic_Model_19078244729512.

Computes per-sample "returning rate" vectors p1, p2 from a [B, 25] grid
(reshaped [B, 5, 5]) of probabilities plus a mask tensor.

Sharding: pure data parallel over the batch. Each of the 8 cores gets
250112 rows (= 128 partitions x 1954); the global batch of 2,000,000 is
zero-padded by 896 rows so every core runs the same SPMD program.

Per-core layout: tiles of [128 partitions, F rows/partition], inputs DMA'd
as contiguous per-partition chunks (row-major [f, c] with c = 0..24 the
5x5 grid). Intermediates live in "k-major" F-blocks so every vector op
processes all F samples of a partition for several grid terms at once:

  prod tile (16 blocks):  T_k = p(4-k, k) * p(4-k, j) at block 4k + (j-1),
  valid j = k+1..4; invalid blocks zeroed so p1_j = sum_k P[4k + j-1]
  becomes two shifted vector adds.

  qs tile (9 blocks): [q40 q31 q22 q13 q04 | S1 S2 S3 S4] with
  q = 1-p (ACT engine), S_j = cumprod of q's; then
  p2_j = Q_j * (1 - S_j) * m_j via two fused scalar_tensor_tensor ops.
"""

import numpy as np

_B = 2_000_000
_NCORES = 8
_FTOT = 1954             # rows per partition per core
_NPC = 128 * _FTOT       # 250112 rows per core
_FMAX = 224              # tile rows per partition


def _legalize_waits(nc):
    """Split multi-wait sync_info into standalone EventSemaphore waits.

    The walrus build in this container encodes at most one sync-wait command
    per ISA instruction ("Too many sync wait commands" otherwise); hoist all
    but the last wait of each instruction into preceding single-wait
    EventSemaphore ops on the same engine (semantically identical: all waits
    are monotone semaphore conditions checked before issue).
    """
    import concourse.mybir as mybir

    for fn in nc.m.functions:
        for blk in fn.blocks:
            out = []
            for inst in blk.instructions:
                si = getattr(inst, "sync_info", None)
                waits = list(si.on_wait) if si is not None and si.on_wait else []
                if len(waits) > 1:
                    for k, w in enumerate(waits[:-1]):
                        out.append(mybir.InstEventSemaphore(
                            name=f"{inst.name}-w{k}",
                            engine=inst.engine,
                            ins=[], outs=[],
                            sync_info=mybir.SyncInfo(on_wait=[w], on_update=[]),
                        ))
                    inst.sync_info = mybir.SyncInfo(
                        on_wait=[waits[-1]],
                        on_update=list(si.on_update) if si.on_update else [],
                    )
                out.append(inst)
            blk.instructions = out
    return nc


def build_nc(ftot=_FTOT, fmax=_FMAX, bufs=2, legalize=True):
    import concourse.bass as bass
    import concourse.mybir as mybir
    from concourse.tile import TileContext

    f32 = mybir.dt.float32
    MUL = mybir.AluOpType.mult
    ADD = mybir.AluOpType.add
    SUB = mybir.AluOpType.subtract
    COPY = mybir.ActivationFunctionType.Copy

    nrows = 128 * ftot
    nc = bass.Bass("TRN2", target_bir_lowering=False, debug=False)
    x = nc.dram_tensor("output", [nrows, 25], f32, kind="ExternalInput")
    mm = nc.dram_tensor("label_mask", [nrows, 25], f32, kind="ExternalInput")
    o1 = nc.dram_tensor("p1", [nrows, 5], f32, kind="ExternalOutput")
    o2 = nc.dram_tensor("p2", [nrows, 5], f32, kind="ExternalOutput")

    with TileContext(nc) as tc:
        with (
            tc.tile_pool(name="io", bufs=bufs) as io,
            tc.tile_pool(name="tmp", bufs=bufs) as tmp,
        ):
            base = 0
            while base < ftot:
                F = min(fmax, ftot - base)
                R0, RN = 128 * base, 128 * F
                x_sl = x[R0:R0 + RN, :].rearrange("(p f) c -> p (f c)", p=128)
                m_sl = mm[R0:R0 + RN, :].rearrange("(p f) c -> p (f c)", p=128)
                o1_sl = o1[R0:R0 + RN, :].rearrange("(p f) c -> p (f c)", p=128)
                o2_sl = o2[R0:R0 + RN, :].rearrange("(p f) c -> p (f c)", p=128)

                tin = io.tile([128, F * 25], f32, tag="tin")
                nc.sync.dma_start(tin[:], x_sl)
                tmk = io.tile([128, F * 25], f32, tag="tmk")
                nc.sync.dma_start(tmk[:], m_sl)

                xin = tin[:].rearrange("p (f c) -> p f c", c=25)
                msk = tmk[:].rearrange("p (f c) -> p f c", c=25)
                mdiag = msk[:, :, 16:0:-4]          # m31 m22 m13 m04

                prod = tmp.tile([128, 16 * F], f32, tag="prod")
                pv = prod[:].rearrange("p (k f) -> p f k", f=F)
                qs = tmp.tile([128, 9 * F], f32, tag="qs")
                qv = qs[:].rearrange("p (k f) -> p f k", f=F)
                t1 = io.tile([128, F * 5], f32, tag="t1")
                o1v = t1[:].rearrange("p (f c) -> p f c", c=5)
                t2 = io.tile([128, F * 5], f32, tag="t2")
                o2v = t2[:].rearrange("p (f c) -> p f c", c=5)

                # ---- p1 ----
                # zero the padding blocks {4, 8, 9, 12, 13, 14}
                nc.gpsimd.memset(pv[:, :, 4:5], 0.0)
                nc.gpsimd.memset(pv[:, :, 8:10], 0.0)
                nc.gpsimd.memset(pv[:, :, 12:15], 0.0)
                # products T_k: (broadcast anchor col) * (col range)
                for bc_c, c0, c1, blk in (
                    (20, 21, 25, 0),    # p40 * p41..p44  -> blocks 0..3
                    (16, 17, 20, 5),    # p31 * p32..p34  -> blocks 5..7
                    (12, 13, 15, 10),   # p22 * p23..p24  -> blocks 10..11
                    (8, 9, 10, 15),     # p13 * p14       -> block  15
                ):
                    n = c1 - c0
                    nc.vector.tensor_tensor(
                        pv[:, :, blk:blk + n],
                        xin[:, :, c0:c1],
                        xin[:, :, bc_c].broadcast_to((128, F, n)),
                        MUL,
                    )
                # mask the leading product of each chain: blocks {0,5,10,15}
                nc.vector.tensor_tensor(
                    pv[:, :, 0:16:5], pv[:, :, 0:16:5], mdiag, MUL
                )
                # p1_j = sum_k P[4k + j-1]: two shifted adds
                nc.vector.tensor_tensor(
                    pv[:, :, 0:8], pv[:, :, 0:8], pv[:, :, 8:16], ADD
                )
                nc.vector.tensor_tensor(
                    o1v[:, :, 1:5], pv[:, :, 0:4], pv[:, :, 4:8], ADD
                )
                nc.gpsimd.memset(o1v[:, :, 0], 0.0)

                # ---- p2 ----
                # q blocks 0..4 = 1 - [p40 p31 p22 p13 p04]  (ACT engine)
                nc.scalar.activation(
                    qv[:, :, 0:5], xin[:, :, 20:0:-4], COPY, bias=1.0, scale=-1.0
                )
                nc.scalar.activation(qv[:, :, 5:6], qv[:, :, 0:1], COPY)
                nc.vector.tensor_tensor(
                    qv[:, :, 6:7], qv[:, :, 5:6], qv[:, :, 1:2], MUL
                )
                nc.vector.tensor_tensor(
                    qv[:, :, 7:8], qv[:, :, 6:7], qv[:, :, 2:3], MUL
                )
                nc.vector.tensor_tensor(
                    qv[:, :, 8:9], qv[:, :, 7:8], qv[:, :, 3:4], MUL
                )
                # W_j = (S_j - 1) * Q_j   (in place over S blocks)
                nc.vector.scalar_tensor_tensor(
                    qv[:, :, 5:9], qv[:, :, 5:9], 1.0, qv[:, :, 1:5], SUB, MUL
                )
                # p2_j = (-W_j) * m_j
                nc.vector.scalar_tensor_tensor(
                    o2v[:, :, 1:5], qv[:, :, 5:9], -1.0, mdiag, MUL, MUL
                )
                nc.gpsimd.memset(o2v[:, :, 0], 0.0)

                nc.sync.dma_start(o1_sl, t1[:])
                nc.sync.dma_start(o2_sl, t2[:])
                base += F
    return _legalize_waits(nc) if legalize else nc


def _run(output, label_mask, **spmd_kwargs):
    from concourse.bass_utils import run_bass_kernel_spmd

    output = np.ascontiguousarray(np.asarray(output), dtype=np.float32)
    label_mask = np.ascontiguousarray(np.asarray(label_mask), dtype=np.float32)
    assert output.shape == (_B, 25) and label_mask.shape == (_B, 25)

    pad = _NCORES * _NPC - _B
    xp = np.concatenate([output, np.zeros((pad, 25), np.float32)], axis=0)
    mp = np.concatenate([label_mask, np.zeros((pad, 25), np.float32)], axis=0)

    in_maps = [
        {
            "output": xp[i * _NPC:(i + 1) * _NPC],
            "label_mask": mp[i * _NPC:(i + 1) * _NPC],
        }
        for i in range(_NCORES)
    ]
    nc = build_nc()
    bres = run_bass_kernel_spmd(nc, in_maps, list(range(_NCORES)), **spmd_kwargs)
    res = bres.results
    p1 = np.concatenate([np.asarray(r["p1"]) for r in res], axis=0)[:_B]
    p2 = np.concatenate([np.asarray(r["p2"]) for r in res], axis=0)[:_B]
    return p1, p2, bres


def kernel(output, label_mask):
    p1, p2, _ = _run(output, label_mask)
    return p1, p2
